# revision 25
# baseline (speedup 1.0000x reference)
"""Trainium2 Bass kernel: memory-slot cross-attention (nn_LocalConstructorMulti).

Reference computation (per batch b):
    Q  = memory_slots @ Wq.T                      [slots, BD]    (shared over b)
    K  = hs_b @ Wk.T ; V = hs_b @ Wv.T            [S, BD]
    s  = (Q_h . K_h) / sqrt(HD)  + mask           [heads, slots, S]
    p  = softmax(s, axis=S);  o = p @ V_h;  y = concat_h(o) @ Wo.T

Key algebraic reassociation (8x FLOP cut vs computing K/V):
    s_h  = (Q_h Wk_h / sqrt(HD)) @ hs.T     -- fold Q@Wk into a tiny [64, HID]
                                               matrix QW on the host
    z    = exp(s + maskbias) @ hs           -- [64, HID] unnormalized context
    d    = exp(s + maskbias) @ 1            -- softmax denominators [64]
    y    = per-head (z/d) @ Wv_h.T @ Wo_h.T -- tiny, done on host (0.15% of
                                               the FLOPs, exact same math)

The device only does the two passes over hs (the 256 MB tensor): scores
(contract HID, consumes hsT) and z (contract rows, consumes hs natural,
obtained via PE transposes of the resident hsT chunks).  Per core this is
~2.2 GFLOP + one 16.8 MB HBM read -- vs 17.2 GFLOP for the naive K/V path.

Sharding: 8 cores = 4 batches x 2 row-halves (2048 rows each).  Softmax
needs no cross-core combine: each core emits unnormalized (z, d) partials
and the host sums them (linear), then normalizes and projects.

Device layout (per core):
  - hsT [HID, 2048] bf16 streamed in 8 chunks of [HID, 256] (one DMA each,
    512B descriptors).  Chunks serve BOTH paths and are then discarded.
  - scores accumulate transposed, sT [row, 64(head*slot)], so rows sit on
    partitions: the additive mask is a per-partition bias fused into the
    Exp activation, and exp output pT feeds the z matmuls directly.
  - z path per chunk: PE-transpose hsT blocks [128,128] -> PSUM (4 blocks
    share a [128,512] tile), drain to SBUF with DVE/ACT alternating, then
    matmul zT[:,m,:] += hs_nat_block.T-free @ pT (PSUM accumulation across
    all 16 row-tiles of the half).
  - d via ones-column matmuls into a [1,64] PSUM accumulator.
  - zT [128,32,64] f32 and d [1,64] DMA straight from PSUM to DRAM.
"""

import sys

if "/opt/trn_rl_repo" not in sys.path:
    sys.path.insert(0, "/opt/trn_rl_repo")

import ml_dtypes
import numpy as np

import concourse.bass as bass  # noqa: F401  (AP helpers)
import concourse.mybir as mybir
import concourse.tile as tile
from concourse import bacc
from concourse.bass_utils import run_bass_kernel_spmd
from concourse.masks import make_identity

BF16 = mybir.dt.bfloat16
F32 = mybir.dt.float32
npbf16 = ml_dtypes.bfloat16

B, S, HID = 4, 4096, 4096
SLOTS, HEADS, BD = 8, 8, 512
HD = BD // HEADS  # 64
NH = HEADS * SLOTS  # 64 score rows (head-major: hn = h*SLOTS + n)
N_CORES = 8
HALVES = N_CORES // B  # row-halves per batch
SH = S // HALVES  # rows per core = 2048
MASK_NEG = -30000.0
SCALE = 1.0 / float(np.sqrt(HD))

CHUNK = 256  # rows per streamed chunk
NCH = SH // CHUNK  # 8 chunks
RPC = CHUNK // 128  # row-tiles per chunk = 2
NRT = SH // 128  # row-tiles per core = 16
NK = HID // 128  # contraction k-tiles = 32

# test.py can flip this to capture an NTFF profile; harness never touches it.
TRACE = False
TRACE_CORES = None
LAST_RESULT = None

_cache = {}


def _build_module():
    """Emit + compile the single-core Bass module (same NEFF on all cores)."""
    nc = bacc.Bacc("TRN2", target_bir_lowering=False, debug=False, num_devices=N_CORES)

    hsT = nc.dram_tensor("hsT", [HID, SH], BF16, kind="ExternalInput").ap()
    qwT = nc.dram_tensor("qwT", [128, NK, NH], BF16, kind="ExternalInput").ap()
    mbT = nc.dram_tensor("mbT", [128, NRT], F32, kind="ExternalInput").ap()
    zS = nc.dram_tensor("zS", [2, 128, NK, NH], BF16, kind="ExternalOutput").ap()
    pS = nc.dram_tensor("pS", [128, NRT, NH], BF16, kind="ExternalOutput").ap()

    hsT_r = hsT.rearrange("(ko ki) n -> ki ko n", ki=128)  # [128, NK, SH]

    with tile.TileContext(nc) as tc:
        with (
            tc.tile_pool(name="consts", bufs=1) as consts,
            tc.tile_pool(name="c0p", bufs=1) as c0p,
            tc.tile_pool(name="hsp", bufs=7) as hsp,
            tc.tile_pool(name="hnp", bufs=4) as hnp,
            tc.tile_pool(name="zps", bufs=1, space="PSUM") as zps,
            tc.tile_pool(name="sps", bufs=2, space="PSUM") as sps,
            tc.tile_pool(name="tps", bufs=2, space="PSUM") as tps,
        ):
            NKH = NK // 2  # k-tiles per DMA half
            NKQ = NK // 4  # k-tiles per chunk-0 quarter
            NOCT = NK // 8  # 4 transpose-octets per row-tile

            # ---- chunk 0 streams in 4 quarter-DMAs: the PE starts
            # pre-transposing octet 0 after ~1.5us instead of ~6us ---------
            c0q = []

            def _c0_quarter(qd):
                tq = c0p.tile([128, NKQ, CHUNK], BF16, tag=f"hsq{qd}")
                nc.sync.dma_start(
                    out=tq, in_=hsT_r[:, qd * NKQ : (qd + 1) * NKQ, 0:CHUNK]
                )
                c0q.append(tq)

            _c0_quarter(0)
            _c0_quarter(1)
            # ---- resident constants (interleaved into the c0 stream) -----
            qw_sb = consts.tile([128, NK, NH], BF16)
            nc.sync.dma_start(out=qw_sb, in_=qwT)
            mb_sb = consts.tile([128, NRT], F32)
            nc.sync.dma_start(out=mb_sb, in_=mbT)
            _c0_quarter(2)
            _c0_quarter(3)
            ident = consts.tile([128, 128], BF16)
            make_identity(nc, ident)

            pt_sb = consts.tile([128, NRT, NH], BF16)  # exp(scores).T rows
            za_sb = consts.tile([128, NK, NH], BF16)  # zT partial rt 0-7
            zb_sb = consts.tile([128, NK, NH], BF16)  # zT partial rt 8-15

            # ---- persistent PSUM accumulator -----------------------------
            z_ps = zps.tile([128, NK, NH], F32)  # zT accumulated over rows

            for c in range(NCH):
                cols = slice(c * CHUNK, (c + 1) * CHUNK)
                if c == 0:
                    def hs_k(k):
                        return (c0q[k // NKQ], k % NKQ)
                else:
                    # two half-DMAs: scores start after the first lands
                    hs_a = hsp.tile([128, NKH, CHUNK], BF16, tag="hsa")
                    nc.sync.dma_start(out=hs_a, in_=hsT_r[:, :NKH, cols])
                    hs_b = hsp.tile([128, NKH, CHUNK], BF16, tag="hsb")
                    nc.sync.dma_start(out=hs_b, in_=hsT_r[:, NKH:, cols])

                    def hs_k(k, hs_a=hs_a, hs_b=hs_b):
                        return (hs_a, k) if k < NKH else (hs_b, k - NKH)

                def _emit_t(q, i, eng):
                    t_ps = tps.tile([128, 1024], BF16, tag="t")
                    for j in range(8):
                        src, kk = hs_k(q * 8 + j)
                        nc.tensor.transpose(
                            t_ps[:, j * 128 : (j + 1) * 128],
                            src[:, kk, i * 128 : (i + 1) * 128],
                            ident,
                        )
                    hn_sb = hnp.tile([128, 1024], BF16, tag="hn")
                    # alternate drain engine so neither becomes the wall
                    if eng == 0:
                        nc.vector.tensor_copy(out=hn_sb, in_=t_ps)
                    else:
                        nc.scalar.copy(out=hn_sb, in_=t_ps)
                    return hn_sb

                # chunk 0: pre-transpose i=0 octets while qw/scores pend
                pre = (
                    [_emit_t(q, 0, int(q == 1)) for q in range(NOCT)]
                    if c == 0
                    else None
                )

                # -- scores sT[row, hn], accumulated over all NK k-tiles ---
                # st tile is one PSUM bank: start=True clears has-written
                # bits BANK-wide, so only the chronologically first matmul
                # may set it; other windows lazily overwrite via cleared bits.
                st_ps = sps.tile([128, RPC, NH], F32, tag="st")
                for k in range(NK):
                    src, kk = hs_k(k)
                    for i in range(RPC):
                        nc.tensor.matmul(
                            st_ps[:, i, :],
                            src[:, kk, i * 128 : (i + 1) * 128],
                            qw_sb[:, k, :],
                            start=(k == 0 and i == 0),
                            stop=(k == NK - 1 and i == RPC - 1),
                        )
                # -- exp with fused per-row mask bias -> pT ----------------
                for i in range(RPC):
                    rt = c * RPC + i
                    nc.scalar.activation(
                        out=pt_sb[:, rt, :],
                        in_=st_ps[:, i, :],
                        func=mybir.ActivationFunctionType.Exp,
                        bias=mb_sb[:, rt : rt + 1],
                        scale=1.0,
                    )
                # stream out pT halves as they complete (overlaps z phase)
                if c == NCH // 2 - 1:
                    nc.sync.dma_start(
                        out=pS[:, : NRT // 2, :], in_=pt_sb[:, : NRT // 2, :]
                    )
                elif c == NCH - 1:
                    nc.sync.dma_start(
                        out=pS[:, NRT // 2 :, :], in_=pt_sb[:, NRT // 2 :, :]
                    )

                # -- z path: transpose hs blocks, then zT += hs_nat.p ------
                # Octets of 8 hid-tiles share one [128,1024] PSUM tile and
                # one drain copy; z matmuls trail the transposes by two
                # octets so the PE rides out the drain latency.
                for i in range(RPC):
                    rt = c * RPC + i

                    def _emit_z(q, hn_sb):
                        # each octet q is one PSUM bank: j==0 at rt==0 is the
                        # bank's first write and carries the lone start=True
                        for j in range(8):
                            nc.tensor.matmul(
                                z_ps[:, q * 8 + j, :],
                                hn_sb[:, j * 128 : (j + 1) * 128],
                                pt_sb[:, rt, :],
                                start=(rt % (NRT // 2) == 0 and j == 0),
                                stop=(rt % (NRT // 2) == NRT // 2 - 1 and j == 7),
                            )

                    if c == 0 and i == 0:
                        for q in range(NOCT):
                            _emit_z(q, pre[q])
                    elif rt == NRT - 1:
                        # final row-tile: transpose all octets up front, then
                        # let each z octet's PSUM drain + output DMA trail it
                        hn_tiles = [
                            _emit_t(q, i, int(q == 1)) for q in range(NOCT)
                        ]
                        for q in range(NOCT):
                            _emit_z(q, hn_tiles[q])
                            sl = slice(q * 8, (q + 1) * 8)
                            if q % 2 == 0:
                                nc.scalar.copy(
                                    out=zb_sb[:, sl, :], in_=z_ps[:, sl, :]
                                )
                            else:
                                nc.vector.tensor_copy(
                                    out=zb_sb[:, sl, :], in_=z_ps[:, sl, :]
                                )
                            if q == 1:
                                nc.sync.dma_start(
                                    out=zS[1][:, : NK // 2, :],
                                    in_=zb_sb[:, : NK // 2, :],
                                )
                            if q == NOCT - 1:
                                nc.sync.dma_start(
                                    out=zS[1][:, NK // 2 :, :],
                                    in_=zb_sb[:, NK // 2 :, :],
                                )
                    else:
                        hn_tiles = [
                            _emit_t(0, i, 0),
                            _emit_t(1, i, 1),
                        ]
                        for q in range(NOCT):
                            if q + 2 < NOCT:
                                hn_tiles.append(_emit_t(q + 2, i, 0))
                            _emit_z(q, hn_tiles[q])

                if c == NCH // 2 - 1:
                    # drain + stream out partial A while chunk 4 scores run
                    # (ACT/DVE are idle during the scores phase)
                    nc.scalar.copy(
                        out=za_sb[:, : NK // 2, :], in_=z_ps[:, : NK // 2, :]
                    )
                    nc.vector.tensor_copy(
                        out=za_sb[:, NK // 2 :, :], in_=z_ps[:, NK // 2 :, :]
                    )
                    nc.sync.dma_start(out=zS[0], in_=za_sb)

    nc.compile()
    return nc


def _get_module():
    if "m" not in _cache:
        _cache["m"] = _build_module()
    return _cache["m"]


def _prep_in_maps(hs, mask, ms, Wq, Wk):
    """Shard the full inputs into 8 per-core input maps (host-side)."""
    # QW[hn, :] = (Q_h / sqrt(HD)) @ Wk_h   with Q = ms @ Wq.T
    Q = (ms @ Wq.T).astype(np.float32)  # [slots, BD]
    Qh = Q.reshape(SLOTS, HEADS, HD)  # [n, h, d]
    Wk3 = Wk.reshape(HEADS, HD, HID)  # [h, d, i]
    QW = np.einsum("nhd,hdi->hni", Qh, Wk3) * np.float32(SCALE)  # [h, n, i]
    qw2 = QW.reshape(NH, HID)  # hn = h*SLOTS + n
    # pack for [128, NK, NH] sbuf layout: qw_p[p, k, j] = qw2[j, k*128+p]
    qw_p = np.ascontiguousarray(
        qw2.T.reshape(NK, 128, NH).transpose(1, 0, 2).astype(npbf16)
    )

    in_maps = []
    for core in range(N_CORES):
        b, g = core // HALVES, core % HALVES
        rows = slice(g * SH, (g + 1) * SH)
        hsT = np.ascontiguousarray(hs[b].T[:, rows].astype(npbf16))
        bias = np.where(mask[b, rows] == 0, np.float32(MASK_NEG), np.float32(0.0))
        mb = np.ascontiguousarray(bias.reshape(NRT, 128).T.astype(np.float32))
        in_maps.append({"hsT": hsT, "qwT": qw_p, "mbT": mb})
    return in_maps


def time_device(inputs_np, reps=8, chain=32):
    """Dev-only helper (not used by grading): estimate per-exec device time
    from the slope of chained async executions with device-resident inputs
    (single-exec wall time is dominated by axon RPC overhead)."""
    import time

    import jax
    from jax.experimental.shard_map import shard_map
    from jax.sharding import Mesh, NamedSharding, PartitionSpec

    import concourse.mybir as mybir_
    from concourse import bass2jax

    nc = _get_module()
    in_maps = _prep_in_maps(
        np.asarray(inputs_np["hidden_states"], np.float32),
        np.asarray(inputs_np["attention_mask"]),
        np.asarray(inputs_np["memory_slots"], np.float32),
        np.asarray(inputs_np["Wq"], np.float32),
        np.asarray(inputs_np["Wk"], np.float32),
    )
    bass2jax.install_neuronx_cc_hook()

    in_names, out_names, out_avals, zero_outs = [], [], [], []
    has_partition = False
    for alloc in nc.m.functions[0].allocations:
        if not isinstance(alloc, mybir_.MemoryLocationSet):
            continue
        name = alloc.memorylocations[0].name
        if alloc.kind == "ExternalInput":
            if name == "partition_id":
                has_partition = True
                continue
            in_names.append(name)
        elif alloc.kind == "ExternalOutput":
            out_names.append(name)
            shape = tuple(alloc.tensor_shape)
            dtype = mybir_.dt.np(alloc.dtype)
            out_avals.append(jax.core.ShapedArray(shape, dtype))
            zero_outs.append(np.zeros(shape, dtype))
    n_params = len(in_names)
    n_outs = len(out_avals)
    # Order must match run_bass_via_pjrt: inputs, donated outputs, partition
    # LAST (neuronx_cc_hook's parameter-order check strips operand[-1]).
    all_names = in_names + out_names + (["partition_id"] if has_partition else [])

    def _body(*args):
        operands = list(args)
        if has_partition:
            operands.append(bass2jax.partition_id_tensor())
        outs = bass2jax._bass_exec_p.bind(
            *operands,
            out_avals=tuple(out_avals),
            in_names=tuple(all_names),
            out_names=tuple(out_names),
            lowering_input_output_aliases=(),
            sim_require_finite=True,
            sim_require_nnan=True,
            nc=nc,
        )
        return tuple(outs)

    devices = jax.devices()[:N_CORES]
    mesh = Mesh(np.asarray(devices), ("core",))
    spec = PartitionSpec("core")
    sharded = jax.jit(
        shard_map(
            _body,
            mesh=mesh,
            in_specs=(spec,) * (n_params + n_outs),
            out_specs=(spec,) * n_outs,
            check_rep=False,
        ),
        donate_argnums=tuple(range(n_params, n_params + n_outs)),
        keep_unused=True,
    )
    concat_in = [
        np.concatenate([np.asarray(in_maps[c][nm]) for c in range(N_CORES)], axis=0)
        for nm in in_names
    ]
    sh = NamedSharding(mesh, spec)
    dev_in = [jax.device_put(a, sh) for a in concat_in]
    jax.block_until_ready(dev_in)

    def _run_chain(n):
        """Issue n executes back-to-back (async dispatch), block once."""
        dz_sets = []
        for _ in range(n):
            zeros = [np.zeros((N_CORES * z.shape[0], *z.shape[1:]), z.dtype)
                     for z in zero_outs]
            dz_sets.append([jax.device_put(z, sh) for z in zeros])
        jax.block_until_ready(dz_sets)
        t0 = time.perf_counter()
        outs = [sharded(*dev_in, *dz) for dz in dz_sets]
        jax.block_until_ready(outs)
        return time.perf_counter() - t0

    _run_chain(1)  # warm compile + caches
    times = []
    for _ in range(reps):
        t1 = _run_chain(1)
        tn = _run_chain(1 + chain)
        times.append((tn - t1) / chain)
    return times


def kernel(hidden_states, attention_mask, memory_slots, Wq, Wk, Wv, Wo):
    global LAST_RESULT
    hs = np.asarray(hidden_states, dtype=np.float32)
    mask = np.asarray(attention_mask)
    ms = np.asarray(memory_slots, dtype=np.float32)
    Wq = np.asarray(Wq, dtype=np.float32)
    Wk = np.asarray(Wk, dtype=np.float32)
    Wv = np.asarray(Wv, dtype=np.float32)
    Wo = np.asarray(Wo, dtype=np.float32)

    nc = _get_module()
    in_maps = _prep_in_maps(hs, mask, ms, Wq, Wk)

    kwargs = {}
    if TRACE:
        kwargs = {"trace": True}
        if TRACE_CORES is not None:
            kwargs["trace_cores"] = TRACE_CORES
    res = run_bass_kernel_spmd(nc, in_maps, core_ids=list(range(N_CORES)), **kwargs)
    LAST_RESULT = res

    # ---- host gather + tiny tail projections (exact same math) ----------
    WvT3 = Wv.reshape(HEADS, HD, HID).transpose(0, 2, 1)  # [h, i, d]
    y = np.empty((B, SLOTS, HID), dtype=np.float32)
    for b in range(B):
        z2 = np.zeros((NH, HID), dtype=np.float32)
        d = np.zeros((NH,), dtype=np.float32)
        for g in range(HALVES):
            r = res.results[b * HALVES + g]
            # zS[p, k, hn] -> z[hn, k*128+p]
            zsum = r["zS"][0].astype(np.float32) + r["zS"][1].astype(np.float32)
            z2 += zsum.transpose(2, 1, 0).reshape(NH, HID)
            d += r["pS"].astype(np.float32).sum(axis=(0, 1))
        o = z2 / d[:, None]  # [hn, HID] attn-weighted mean of hs rows
        o3 = o.reshape(HEADS, SLOTS, HID)
        ov = np.matmul(o3, WvT3)  # [h, n, d]
        ovR = ov.transpose(1, 0, 2).reshape(SLOTS, BD)  # [n, h*d]
        y[b] = ovR @ Wo.T
    return np.ascontiguousarray(y)
